# revision 2
# baseline (speedup 1.0000x reference)
"""APPNP (2-layer MLP + K-step personalized-pagerank propagation) on 8 TRN2 NeuronCores.

Strategy
--------
z = MLP(x);  for k in 1..K: z <- (1-a)*A_hat z + a*h,  A_hat = D^-1/2 (A+I) D^-1/2.

The GCN norm factors: norm_e = dinv[src]*dinv[dst], so with u = dinv * z the
per-edge work is a pure gather+sum:  agg[d] = sum_{e->d} u[src_e];
z' = (1-a)*dinv[d]*agg[d] + a*h[d].

Sharding: destination nodes are split contiguously across the 8 cores.  Each
step, every core row-scales its z-shard into u, the shards are AllGathered
into a replicated HBM table (bf16 rows padded to 256B), and each core
dma_gathers the source rows for its edges.  Edges are bucketed on the host by
(dst-block-of-128, src-quarter) — the quarter split keeps gather indices in
int16 range — sorted so each bucket's slots are contiguous and padded to a
multiple of 128.  A 128-slot tile is reduced onto its 128-dst block with a
one-hot matrix (built on DVE via iota==dst_local) as the PE matmul weights,
accumulating in PSUM; the four quarter passes accumulate in an SBUF fp32
buffer.  The bucket structure is the max over cores so all 8 cores run one
SPMD graph.
"""
import sys
for _p in ("/opt/trn_rl_repo",):
    if _p not in sys.path:
        sys.path.insert(0, _p)

import os as _os
# RDH collective algorithm (1-56MB messages) has a known hang mode; force it off
# before jax/NRT initializes.
_os.environ.setdefault("NEURON_RT_DBG_RDH_CC", "0")

import math
import os
import numpy as np
import ml_dtypes

import concourse.bass as bass
import concourse.bacc as bacc
import concourse.tile as tile
from concourse import mybir
from concourse.bass_utils import run_bass_kernel_spmd

NCORES = 8
K_STEPS = 5
ALPHA = 0.1
FP = 128          # padded bf16 row elems (256 bytes)
CHUNK_TILES = 96  # gather chunk = 96 tiles = 12288 slots (3MB bf16)

bf16 = mybir.dt.bfloat16
f32 = mybir.dt.float32
i16 = mybir.dt.int16
AOP = mybir.AluOpType
ACT = mybir.ActivationFunctionType

_BF16 = ml_dtypes.bfloat16


def _wrap16(a):
    """slot i -> (partition i%16, col i//16); replicated to all 128 partitions
    so any SWDGE queue's Q7 pair (partitions 32q..32q+31) can read them."""
    w = a.reshape(-1, 16).T
    return np.tile(w, (8, 1))


def _host_prep(x, W1, b1, W2, b2, edge_index):
    N, N_IN = x.shape
    N_HID = W1.shape[0]
    F = W2.shape[0]
    assert N % NCORES == 0
    RPC = N // NCORES
    NB = (RPC + 127) // 128
    RPAD = NB * 128
    QROWS = 2 * RPAD

    src = edge_index[0].astype(np.int64)
    dst = edge_index[1].astype(np.int64)
    deg = np.bincount(dst, minlength=N).astype(np.float64) + 1.0
    dinv = (1.0 / np.sqrt(deg)).astype(np.float32)

    # self-loops are NOT materialized as edges: their message is u[dst],
    # which lives in the owning core's SBUF (u_stage) — added on-chip.
    # deg still counts the loop (reference semantics).
    src_all = src
    dst_all = dst

    # table row of global node s (dst-padded layout)
    tblrow_of = (src_all // RPC) * RPAD + (src_all % RPC)
    q_of = tblrow_of // QROWS
    lidx_of = (tblrow_of % QROWS).astype(np.int64)
    assert lidx_of.max() < 32768

    NG = 4 * NB
    owner = dst_all // RPC
    per_core = []
    cnts = np.zeros((NCORES, NG), np.int64)
    for c in range(NCORES):
        m = owner == c
        dloc = dst_all[m] - RPC * c
        key = q_of[m] * NB + (dloc >> 7)
        order = np.argsort(key, kind="stable")
        per_core.append((key[order], lidx_of[m][order], (dloc & 127)[order]))
        cnts[c] = np.bincount(key[order], minlength=NG)

    T_g = np.maximum(1, (cnts.max(axis=0) + 127) // 128)  # tiles per group
    pad_g = T_g * 128
    off_pad = np.concatenate([[0], np.cumsum(pad_g)[:-1]])
    S_total = int(pad_g.sum())
    T_total = int(T_g.sum())

    idx_arrs, dstl_arrs = [], []
    for c in range(NCORES):
        key_s, lidx_s, dl_s = per_core[c]
        gstart = np.concatenate([[0], np.cumsum(cnts[c])[:-1]])
        within = np.arange(len(key_s)) - gstart[key_s]
        pos = off_pad[key_s] + within
        idx_a = np.zeros(S_total, np.int16)
        dstl_a = np.full(S_total, -1.0, np.float32)
        idx_a[pos] = lidx_s.astype(np.int16)
        dstl_a[pos] = dl_s
        # wrap per quarter, concat along cols
        qs = []
        dstl_cols = []
        for q in range(4):
            s0 = int(off_pad[q * NB]) if q * NB < NG else S_total
            s1 = int(off_pad[(q + 1) * NB]) if (q + 1) * NB < NG else S_total
            qs.append(_wrap16(idx_a[s0:s1]))
        idx_arrs.append(np.concatenate(qs, axis=1))
        dstl_arrs.append(dstl_a.reshape(-1, 128).T.astype(_BF16))  # [128, T_total]

    # tile metadata, shared by all cores: stream order (q, b, tile)
    quarter_tiles = []  # per q: list of (b, start, stop)
    for q in range(4):
        lst = []
        for b in range(NB):
            T = int(T_g[q * NB + b])
            for t in range(T):
                lst.append((b, t == 0, t == T - 1))
        quarter_tiles.append(lst)

    # MLP / scalar inputs
    xT = np.zeros((N_IN, NCORES * RPAD), _BF16)
    xv = x.astype(_BF16)
    for c in range(NCORES):
        xT[:, c * RPAD:c * RPAD + RPC] = xv[c * RPC:(c + 1) * RPC].T
    w1t = np.ascontiguousarray(W1.T).astype(_BF16)   # [N_IN, N_HID]
    w2t = np.ascontiguousarray(W2.T).astype(_BF16)   # [N_HID, F]
    b1c = np.ascontiguousarray(b1.reshape(N_HID // 128, 128).T).astype(np.float32)
    b2b01 = np.tile(b2.astype(np.float32) * ALPHA, (128, 1)).astype(np.float32)

    dinv_pad = np.ones(NCORES * RPAD, np.float32)
    for c in range(NCORES):
        dinv_pad[c * RPAD:c * RPAD + RPC] = dinv[c * RPC:(c + 1) * RPC]
    dnv = dinv_pad.reshape(NCORES, NB, 128)
    scu = ((1.0 - ALPHA) * dnv * dnv).astype(np.float32)
    scz = ((1.0 - ALPHA) * dnv).astype(np.float32)

    iota8 = np.broadcast_to(np.arange(128, dtype=np.float32), (128, 8, 128)).astype(_BF16)

    meta = dict(N=N, N_IN=N_IN, N_HID=N_HID, F=F, RPC=RPC, NB=NB, RPAD=RPAD,
                QROWS=QROWS, T_total=T_total, S_total=S_total,
                quarter_tiles=quarter_tiles)
    in_maps = []
    for c in range(NCORES):
        in_maps.append({
            "xT": np.ascontiguousarray(xT[:, c * RPAD:(c + 1) * RPAD]),
            "w1t": w1t, "w2t": w2t, "b1c": b1c, "b2b01": b2b01,
            "idx": np.ascontiguousarray(idx_arrs[c]),
            "dstl": np.ascontiguousarray(dstl_arrs[c]),
            "scu": np.ascontiguousarray(scu[c].transpose(1, 0)),
            "scz": np.ascontiguousarray(scz[c].transpose(1, 0)),
            "dnv": np.ascontiguousarray(dnv[c].transpose(1, 0)),
            "iota8": np.ascontiguousarray(iota8),
        })
    return meta, in_maps


def _build(meta):
    N_IN, N_HID, F = meta["N_IN"], meta["N_HID"], meta["F"]
    NB, RPAD, QROWS = meta["NB"], meta["RPAD"], meta["QROWS"]
    T_total, S_total = meta["T_total"], meta["S_total"]
    quarter_tiles = meta["quarter_tiles"]
    KI = N_IN // 128   # k tiles for fc1
    KH = N_HID // 128  # k tiles for fc2

    nc = bacc.Bacc("TRN2", target_bir_lowering=False, debug=False,
                   num_devices=NCORES, num_swdge_queues=4)

    xT_in = nc.dram_tensor("xT", [N_IN, RPAD], bf16, kind="ExternalInput")
    w1t_in = nc.dram_tensor("w1t", [N_IN, N_HID], bf16, kind="ExternalInput")
    w2t_in = nc.dram_tensor("w2t", [N_HID, F], bf16, kind="ExternalInput")
    b1c_in = nc.dram_tensor("b1c", [128, KH], f32, kind="ExternalInput")
    b2b01_in = nc.dram_tensor("b2b01", [128, F], f32, kind="ExternalInput")
    idx_in = nc.dram_tensor("idx", [128, S_total // 16], i16, kind="ExternalInput")
    dstl_in = nc.dram_tensor("dstl", [128, T_total], bf16, kind="ExternalInput")
    scu_in = nc.dram_tensor("scu", [128, NB], f32, kind="ExternalInput")
    scz_in = nc.dram_tensor("scz", [128, NB], f32, kind="ExternalInput")
    dnv_in = nc.dram_tensor("dnv", [128, NB], f32, kind="ExternalInput")
    iota8_in = nc.dram_tensor("iota8", [128, 8, 128], bf16, kind="ExternalInput")
    z_out = nc.dram_tensor("z_out", [RPAD, F], f32, kind="ExternalOutput")

    ag_in = nc.dram_tensor("ag_in", [RPAD, FP], bf16, kind="Internal")
    table = nc.dram_tensor("table", [NCORES * RPAD, FP], bf16,
                           kind="Internal", addr_space="Shared")

    with tile.TileContext(nc) as tc:
        with (
            tc.tile_pool(name="const", bufs=1) as cp,
            tc.tile_pool(name="persist", bufs=1) as pp,
        ):
            w1t_sb = cp.tile([128, KI, N_HID], bf16)
            nc.sync.dma_start(w1t_sb[:], w1t_in.ap().rearrange("(k p) h -> p k h", p=128))
            w2t_sb = cp.tile([128, KH, F], bf16)
            nc.sync.dma_start(w2t_sb[:], w2t_in.ap().rearrange("(k p) f -> p k f", p=128))
            b1_sb = cp.tile([128, KH], f32)
            nc.sync.dma_start(b1_sb[:], b1c_in[:])
            b2b01_sb = cp.tile([128, F], f32)
            nc.sync.dma_start(b2b01_sb[:], b2b01_in[:])
            iota8_sb = cp.tile([128, 8, 128], bf16)
            nc.sync.dma_start(iota8_sb[:], iota8_in[:])
            dstl_sb = cp.tile([128, T_total], bf16)
            nc.sync.dma_start(dstl_sb[:], dstl_in[:])
            scu_sb = cp.tile([128, NB], f32)
            nc.sync.dma_start(scu_sb[:], scu_in[:])
            scz_sb = cp.tile([128, NB], f32)
            nc.sync.dma_start(scz_sb[:], scz_in[:])
            dnv_sb = cp.tile([128, NB], f32)
            nc.sync.dma_start(dnv_sb[:], dnv_in[:])

            hc_sb = pp.tile([128, NB, F], bf16)     # alpha*h
            hcu_sb = pp.tile([128, NB, F], bf16)    # alpha*dinv*h
            acc_sb = pp.tile([128, NB, F], f32)     # sum over 4 quarter passes
            u_stage = pp.tile([128, NB, FP], bf16)  # u rows (padded to 256B)
            z_stage = pp.tile([128, NB, F], f32)
            nc.vector.memset(u_stage[:], 0.0)

            # ---------------- MLP ----------------
            with (
                tc.tile_pool(name="xk", bufs=2 * KI + 2) as xp,
                tc.tile_pool(name="h1", bufs=2) as hp,
                tc.tile_pool(name="ps1", bufs=2, space="PSUM") as ps1,
                tc.tile_pool(name="ps2", bufs=2, space="PSUM") as ps2,
            ):
                row_chunks = []
                r = 0
                while r < RPAD:
                    w = min(512, RPAD - r)
                    row_chunks.append((r, w))
                    r += w
                for (r0, cw) in row_chunks:
                    xks = []
                    for k in range(KI):
                        xk = xp.tile([128, 512], bf16, tag="xk")
                        nc.sync.dma_start(xk[:, 0:cw], xT_in.ap()[128 * k:128 * (k + 1), r0:r0 + cw])
                        xks.append(xk)
                    h1 = hp.tile([128, KH, 512], bf16, tag="h1")
                    for h2 in range(KH):
                        psum1 = ps1.tile([128, 512], f32, tag="ps1")
                        for k in range(KI):
                            nc.tensor.matmul(
                                psum1[:, 0:cw],
                                w1t_sb[:, k, 128 * h2:128 * (h2 + 1)],
                                xks[k][:, 0:cw],
                                start=(k == 0), stop=(k == KI - 1),
                            )
                        nc.scalar.activation(h1[:, h2, 0:cw], psum1[:, 0:cw],
                                             ACT.Relu, bias=b1_sb[:, h2:h2 + 1])
                    for rb in range(cw // 128):
                        b = (r0 + 128 * rb) // 128
                        psum2 = ps2.tile([128, F], f32, tag="ps2")
                        for h2 in range(KH):
                            nc.tensor.matmul(
                                psum2[:],
                                h1[:, h2, 128 * rb:128 * (rb + 1)],
                                w2t_sb[:, h2, :],
                                start=(h2 == 0), stop=(h2 == KH - 1),
                            )
                        # hc = alpha*(psum2 + b2) ; hcu = dinv*hc ; u0 = 10*hcu
                        nc.vector.scalar_tensor_tensor(
                            hc_sb[:, b, :], psum2[:], ALPHA, b2b01_sb[:],
                            op0=AOP.mult, op1=AOP.add)
                        nc.vector.tensor_scalar_mul(
                            hcu_sb[:, b, :], hc_sb[:, b, :], dnv_sb[:, b:b + 1])
                        nc.vector.tensor_scalar_mul(
                            u_stage[:, b, 0:F], hcu_sb[:, b, :], 1.0 / ALPHA)

            # ---------------- propagation ----------------
            with (
                tc.tile_pool(name="gth", bufs=2) as gp,
                tc.tile_pool(name="oh", bufs=3) as op_,
                tc.tile_pool(name="idxp", bufs=2) as ip,
                tc.tile_pool(name="psp", bufs=4, space="PSUM") as psp,
            ):
                for step in range(K_STEPS):
                    # u shards -> replicated table
                    nc.sync.dma_start(
                        ag_in.ap().rearrange("(b p) f -> p b f", p=128), u_stage[:])
                    nc.gpsimd.collective_compute(
                        "AllGather", AOP.bypass,
                        replica_groups=[list(range(NCORES))],
                        ins=[ag_in.ap().opt()], outs=[table.ap().opt()],
                    )

                    tglob = 0
                    slot_off = 0  # global slot offset (into idx cols)
                    for q in range(4):
                        tiles_q = quarter_tiles[q]
                        tbl_q = table.ap()[QROWS * q:QROWS * (q + 1), :]
                        ci = 0
                        while ci < len(tiles_q):
                            cn = min(CHUNK_TILES, len(tiles_q) - ci)
                            cs = cn * 128
                            idx_sb = ip.tile([128, CHUNK_TILES * 8], i16, tag="idx")
                            nc.sync.dma_start(
                                idx_sb[:, 0:cs // 16],
                                idx_in.ap()[:, slot_off // 16:(slot_off + cs) // 16])
                            g = gp.tile([128, CHUNK_TILES, FP], bf16, tag="g")
                            # per-tile gathers: >128-idx dma_gather hangs on HW.
                            # round-robin the 4 SWDGE queues so 4 Q7 pairs
                            # generate descriptors concurrently.
                            for t in range(cn):
                                nc.gpsimd.dma_gather(
                                    g[:, t:t + 1, :], tbl_q,
                                    idx_sb[:, t * 8:(t + 1) * 8],
                                    128, 128, FP, queue_num=t % 4)
                            # one-hot groups of 8 tiles
                            for g0 in range(0, cn, 8):
                                gl = min(8, cn - g0)
                                oh = op_.tile([128, 8, 128], bf16, tag="oh")
                                nc.vector.tensor_tensor(
                                    oh[:, 0:gl, :], iota8_sb[:, 0:gl, :],
                                    dstl_sb[:, tglob + g0:tglob + g0 + gl]
                                    .broadcast_to((128, gl, 128)),
                                    AOP.is_equal)
                                for j in range(gl):
                                    b, st, sp = tiles_q[ci + g0 + j]
                                    if st:
                                        psum = psp.tile([128, F], f32, tag="ps")
                                    nc.tensor.matmul(
                                        psum[:], oh[:, j, :], g[:, g0 + j, 0:F],
                                        start=st, stop=sp)
                                    if sp:
                                        if q == 0:
                                            nc.vector.tensor_copy(acc_sb[:, b, :], psum[:])
                                        else:
                                            nc.vector.tensor_tensor(
                                                acc_sb[:, b, :], acc_sb[:, b, :],
                                                psum[:], AOP.add)
                            tglob += cn
                            slot_off += cs
                            ci += cn

                    for b in range(NB):
                        # self-loop contribution: agg += u (local, no gather)
                        nc.vector.tensor_tensor(
                            acc_sb[:, b, :], acc_sb[:, b, :],
                            u_stage[:, b, 0:F], AOP.add)
                        if step < K_STEPS - 1:
                            nc.vector.scalar_tensor_tensor(
                                u_stage[:, b, 0:F], acc_sb[:, b, :],
                                scu_sb[:, b:b + 1], hcu_sb[:, b, :],
                                op0=AOP.mult, op1=AOP.add)
                        else:
                            nc.vector.scalar_tensor_tensor(
                                z_stage[:, b, :], acc_sb[:, b, :],
                                scz_sb[:, b:b + 1], hc_sb[:, b, :],
                                op0=AOP.mult, op1=AOP.add)

                nc.sync.dma_start(
                    z_out.ap().rearrange("(b p) f -> p b f", p=128), z_stage[:])

    nc.compile()
    return nc


def kernel(x, W1, b1, W2, b2, edge_index):
    x = np.asarray(x, dtype=np.float32)
    W1 = np.asarray(W1, dtype=np.float32)
    b1 = np.asarray(b1, dtype=np.float32)
    W2 = np.asarray(W2, dtype=np.float32)
    b2 = np.asarray(b2, dtype=np.float32)
    ei_np = np.asarray(edge_index)

    meta, in_maps = _host_prep(x, W1, b1, W2, b2, ei_np)
    nc = _build(meta)
    trace = os.environ.get("KTRACE", "0") == "1"
    try:
        res = run_bass_kernel_spmd(nc, in_maps, core_ids=list(range(NCORES)),
                                   trace=trace)
    except ModuleNotFoundError:
        res = run_bass_kernel_spmd(nc, in_maps, core_ids=list(range(NCORES)))
    global LAST_EXEC_NS, LAST_TRACE
    LAST_EXEC_NS = res.exec_time_ns
    LAST_TRACE = res
    RPC = meta["RPC"]
    z = np.concatenate([res.results[c]["z_out"][:RPC] for c in range(NCORES)], axis=0)
    return z.astype(np.float32)


if __name__ == "__main__":
    # quick self-test at reduced size
    rng = np.random.default_rng(0)
    N, NI, NH, F, E = 12800, 512, 256, 64, 80000
    x = rng.standard_normal((N, NI), dtype=np.float32)
    W1 = rng.standard_normal((NH, NI), dtype=np.float32) * 0.05
    b1 = rng.standard_normal(NH).astype(np.float32) * 0.1
    W2 = rng.standard_normal((F, NH), dtype=np.float32) * 0.05
    b2 = rng.standard_normal(F).astype(np.float32) * 0.1
    ei = rng.integers(0, N, (2, E)).astype(np.int32)

    z = kernel(x=x, W1=W1, b1=b1, W2=W2, b2=b2, edge_index=ei)

    # numpy reference
    h = np.maximum(x @ W1.T + b1, 0.0) @ W2.T + b2
    deg = np.bincount(ei[1], minlength=N) + 1.0
    dinv = 1.0 / np.sqrt(deg)
    src = np.concatenate([ei[0], np.arange(N)])
    dst = np.concatenate([ei[1], np.arange(N)])
    norm = dinv[src] * dinv[dst]
    zr = h.copy()
    for _ in range(K_STEPS):
        msg = zr[src] * norm[:, None]
        agg = np.zeros_like(zr)
        np.add.at(agg, dst, msg)
        zr = 0.9 * agg + 0.1 * h
    rel = np.linalg.norm(z - zr) / np.linalg.norm(zr)
    print("rel err:", rel)



# revision 3
# speedup vs baseline: 2.0348x; 2.0348x over previous
"""APPNP (2-layer MLP + K-step PPR propagation) on 8 TRN2 NeuronCores — v2.

v2 strategy (vs v0 baseline):
- dma_gather ops are large (up to OP_TILES tiles = 2048 indices per op,
  single_packet=False) instead of 128-idx ops: the Q7 descriptor-generation
  path costs ~8ns/idx per queue-pair and ~0.5us fixed per op, so big ops on
  4 round-robin queues reach ~2ns/idx aggregate (the structural ceiling of
  the SWDGE descriptor-write path) vs ~5ns/idx effective in the baseline.
- One-hot scatter matrices are built in fp8 (exact for 0/1) so LDWEIGHTS
  runs with fp8 fast-weight-load; matmul is fp8 lhsT x bf16 rhs.
- Self-loop folded into the j==0 accumulate (acc = psum + u); single fused
  psum chain per (quarter, dst-block) bucket; z written via scz/hc fusion.

Sharding: dst-nodes contiguous across cores (12500/core); table rows
core-major (node (c,r) -> c*12544+r), int16 gather indices within each
of 4 table quarters.
"""
import sys
for _p in ("/opt/trn_rl_repo",):
    if _p not in sys.path:
        sys.path.insert(0, _p)

import os as _os
_os.environ.setdefault("NEURON_RT_DBG_RDH_CC", "0")

import os
import numpy as np
import ml_dtypes

import concourse.bass as bass
import concourse.bacc as bacc
import concourse.tile as tile
from concourse import mybir
from concourse.bass_utils import run_bass_kernel_spmd

NCORES = 8
K_STEPS = 5
ALPHA = 0.1
FP = 128            # table row elems (bf16) = 256B
OP_TILES = 16       # max tiles per gather op (2048 idx)
GBIG = __import__('os').environ.get("GBIG", "1") == "1"   # big gather ops
TRIM = __import__('os').environ.get("TRIM", "0") == "1"   # -1 pad trim (breaks 8-core SPMD; keep off)
RSLC = [0, 3200, 6400, 9600, 12544]
SZ = [3200, 3200, 3200, 2944]
BSLC = [0, 25, 50, 75, 98]   # u_stage block boundaries per slice

bf16 = mybir.dt.bfloat16
f32 = mybir.dt.float32
fp8 = mybir.dt.float8e4
i16 = mybir.dt.int16
AOP = mybir.AluOpType
ACT = mybir.ActivationFunctionType

_BF16 = ml_dtypes.bfloat16


def _wrap16(a):
    w = a.reshape(-1, 16).T
    return np.tile(w, (8, 1))


def _host_prep(x, W1, b1, W2, b2, edge_index):
    N, N_IN = x.shape
    N_HID = W1.shape[0]
    F = W2.shape[0]
    assert N % NCORES == 0
    RPC = N // NCORES
    NB = (RPC + 127) // 128
    RPAD = NB * 128
    assert RPAD == 12544 and NB == 98

    src = edge_index[0].astype(np.int64)
    dst = edge_index[1].astype(np.int64)
    deg = np.bincount(dst, minlength=N).astype(np.float64) + 1.0
    dinv = (1.0 / np.sqrt(deg)).astype(np.float32)

    QROWS = 2 * RPAD
    tblrow = (src // RPC) * RPAD + (src % RPC)
    j_of = tblrow // QROWS
    lidx = tblrow % QROWS
    assert lidx.max() < 32768

    owner = dst // RPC
    dloc = dst - owner * RPC
    bblk = dloc >> 7
    dl = dloc & 127
    key = j_of * NB + bblk
    NG = 4 * NB

    cnts = np.zeros((NCORES, NG), np.int64)
    percore = []
    for c in range(NCORES):
        m = owner == c
        k_c = key[m]
        order = np.argsort(k_c, kind="stable")
        percore.append((k_c[order], lidx[m][order], dl[m][order]))
        cnts[c] = np.bincount(k_c, minlength=NG)

    maxc = cnts.max(axis=0)
    T_g = (maxc + 127) // 128            # 0 for empty buckets
    S_g = T_g * 128
    off_slot = np.concatenate([[0], np.cumsum(S_g)[:-1]])
    S_total = int(S_g.sum())
    T_total = int(T_g.sum())
    tile_of_bucket = np.concatenate([[0], np.cumsum(T_g)[:-1]])

    # gather op list: per slice j, pack consecutive non-empty buckets
    ops = []   # (j, slot_off, n_slots, buckets:[(key, tile0, ntiles)])
    for j in range(4):
        cur = None
        for b in range(NB):
            kk = j * NB + b
            if T_g[kk] == 0:
                continue
            nt = int(T_g[kk])
            if cur is None or cur["nt"] + nt > OP_TILES:
                if cur is not None:
                    ops.append(cur)
                cur = {"j": j, "slot_off": int(off_slot[kk]), "nt": 0,
                       "buckets": []}
            cur["buckets"].append((kk, int(tile_of_bucket[kk]), nt))
            cur["nt"] += nt
        if cur is not None:
            ops.append(cur)

    # per-core idx/dstl arrays
    idx_arrs, dstl_arrs = [], []
    for c in range(NCORES):
        key_s, lidx_s, dl_s = percore[c]
        gstart = np.concatenate([[0], np.cumsum(cnts[c])[:-1]])
        within = np.arange(len(key_s)) - gstart[key_s]
        pos = off_slot[key_s] + within
        idx_a = np.full(S_total, -1, np.int16)
        dstl_a = np.full(S_total, -1.0, np.float32)
        idx_a[pos] = lidx_s.astype(np.int16)
        dstl_a[pos] = dl_s
        # interior buckets of each op: pads -> 0 (safe row); last bucket
        # keeps -1 so the ucode trims trailing descriptors.
        for op in ops:
            for (kk, _, _) in op["buckets"][:-1]:
                s0, s1 = int(off_slot[kk]), int(off_slot[kk] + S_g[kk])
                seg = idx_a[s0:s1]
                seg[seg < 0] = 0
        if not TRIM:
            idx_a[idx_a < 0] = 0
        idx_arrs.append(_wrap16(idx_a))
        dstl_arrs.append(dstl_a.reshape(-1, 128).T.astype(_BF16))

    # MLP inputs (transposed x per core)
    xT = np.zeros((N_IN, NCORES * RPAD), _BF16)
    xv = x.astype(_BF16)
    for c in range(NCORES):
        xT[:, c * RPAD:c * RPAD + RPC] = xv[c * RPC:(c + 1) * RPC].T
    w1t = np.ascontiguousarray(W1.T).astype(_BF16)
    w2t = np.ascontiguousarray(W2.T).astype(_BF16)
    KH = N_HID // 128
    b1c = np.ascontiguousarray(b1.reshape(KH, 128).T).astype(np.float32)
    b2b01 = np.tile(b2.astype(np.float32) * ALPHA, (128, 1)).astype(np.float32)

    dinv_pad = np.ones(NCORES * RPAD, np.float32)
    for c in range(NCORES):
        dinv_pad[c * RPAD:c * RPAD + RPC] = dinv[c * RPC:(c + 1) * RPC]
    dnv = dinv_pad.reshape(NCORES, NB, 128)
    scu = ((1.0 - ALPHA) * dnv * dnv).astype(np.float32)
    scz = ((1.0 - ALPHA) * dnv).astype(np.float32)

    iota8 = np.broadcast_to(np.arange(128, dtype=np.float32), (128, 8, 128))

    meta = dict(N=N, N_IN=N_IN, N_HID=N_HID, F=F, RPC=RPC, NB=NB, RPAD=RPAD,
                QROWS=QROWS, S_total=S_total, T_total=T_total, ops=ops)
    in_maps = []
    for c in range(NCORES):
        in_maps.append({
            "xT": np.ascontiguousarray(xT[:, c * RPAD:(c + 1) * RPAD]),
            "w1t": w1t, "w2t": w2t, "b1c": b1c, "b2b01": b2b01,
            "idx": np.ascontiguousarray(idx_arrs[c]),
            "dstl": np.ascontiguousarray(dstl_arrs[c]),
            "scu": np.ascontiguousarray(scu[c].transpose(1, 0)),
            "scz": np.ascontiguousarray(scz[c].transpose(1, 0)),
            "dnv": np.ascontiguousarray(dnv[c].transpose(1, 0)),
            "iota8": np.ascontiguousarray(iota8.astype(_BF16)),
        })
    return meta, in_maps


# revision 4
# speedup vs baseline: 2.1178x; 1.0408x over previous
"""APPNP (2-layer MLP + K-step PPR propagation) on 8 TRN2 NeuronCores — v2.

v2 strategy (vs v0 baseline):
- dma_gather ops are large (up to OP_TILES tiles = 2048 indices per op,
  single_packet=False) instead of 128-idx ops: the Q7 descriptor-generation
  path costs ~8ns/idx per queue-pair and ~0.5us fixed per op, so big ops on
  4 round-robin queues reach ~2ns/idx aggregate (the structural ceiling of
  the SWDGE descriptor-write path) vs ~5ns/idx effective in the baseline.
- One-hot scatter matrices are built in fp8 (exact for 0/1) so LDWEIGHTS
  runs with fp8 fast-weight-load; matmul is fp8 lhsT x bf16 rhs.
- Self-loop folded into the j==0 accumulate (acc = psum + u); single fused
  psum chain per (quarter, dst-block) bucket; z written via scz/hc fusion.

Sharding: dst-nodes contiguous across cores (12500/core); table rows
core-major (node (c,r) -> c*12544+r), int16 gather indices within each
of 4 table quarters.
"""
import sys
for _p in ("/opt/trn_rl_repo",):
    if _p not in sys.path:
        sys.path.insert(0, _p)

import os as _os
_os.environ.setdefault("NEURON_RT_DBG_RDH_CC", "0")

import os
import numpy as np
import ml_dtypes

import concourse.bass as bass
import concourse.bacc as bacc
import concourse.tile as tile
from concourse import mybir
from concourse.bass_utils import run_bass_kernel_spmd

NCORES = 8
K_STEPS = 5
ALPHA = 0.1
FP = 64             # table row elems (bf16) = 128B (SBUF-source gather)
OP_TILES = int(__import__('os').environ.get("OPT", "8"))  # tiles per gather op
GBIG = __import__('os').environ.get("GBIG", "1") == "1"   # big gather ops
TRIM = __import__('os').environ.get("TRIM", "0") == "1"   # -1 pad trim (breaks 8-core SPMD; keep off)
RSLC = [0, 3200, 6400, 9600, 12544]
SZ = [3200, 3200, 3200, 2944]
BSLC = [0, 25, 50, 75, 98]   # u_stage block boundaries per slice

bf16 = mybir.dt.bfloat16
f32 = mybir.dt.float32
fp8 = mybir.dt.float8e4
i16 = mybir.dt.int16
AOP = mybir.AluOpType
ACT = mybir.ActivationFunctionType

_BF16 = ml_dtypes.bfloat16


def _wrap16(a):
    w = a.reshape(-1, 16).T
    return np.tile(w, (8, 1))


def _host_prep(x, W1, b1, W2, b2, edge_index):
    N, N_IN = x.shape
    N_HID = W1.shape[0]
    F = W2.shape[0]
    assert N % NCORES == 0
    RPC = N // NCORES
    NB = (RPC + 127) // 128
    RPAD = NB * 128
    assert RPAD == 12544 and NB == 98

    src = edge_index[0].astype(np.int64)
    dst = edge_index[1].astype(np.int64)
    deg = np.bincount(dst, minlength=N).astype(np.float64) + 1.0
    dinv = (1.0 / np.sqrt(deg)).astype(np.float32)

    QROWS = 2 * RPAD
    tblrow = (src // RPC) * RPAD + (src % RPC)
    j_of = tblrow // QROWS
    lidx = tblrow % QROWS
    assert lidx.max() < 32768

    owner = dst // RPC
    dloc = dst - owner * RPC
    bblk = dloc >> 7
    dl = dloc & 127
    key = j_of * NB + bblk
    NG = 4 * NB

    cnts = np.zeros((NCORES, NG), np.int64)
    percore = []
    for c in range(NCORES):
        m = owner == c
        k_c = key[m]
        order = np.argsort(k_c, kind="stable")
        percore.append((k_c[order], lidx[m][order], dl[m][order]))
        cnts[c] = np.bincount(k_c, minlength=NG)

    maxc = cnts.max(axis=0)
    T_g = (maxc + 127) // 128            # 0 for empty buckets
    S_g = T_g * 128
    off_slot = np.concatenate([[0], np.cumsum(S_g)[:-1]])
    S_total = int(S_g.sum())
    T_total = int(T_g.sum())
    tile_of_bucket = np.concatenate([[0], np.cumsum(T_g)[:-1]])

    # gather op list: per slice j, pack consecutive non-empty buckets
    ops = []   # (j, slot_off, n_slots, buckets:[(key, tile0, ntiles)])
    for j in range(4):
        cur = None
        for b in range(NB):
            kk = j * NB + b
            if T_g[kk] == 0:
                continue
            nt = int(T_g[kk])
            if cur is None or cur["nt"] + nt > OP_TILES:
                if cur is not None:
                    ops.append(cur)
                cur = {"j": j, "slot_off": int(off_slot[kk]), "nt": 0,
                       "buckets": []}
            cur["buckets"].append((kk, int(tile_of_bucket[kk]), nt))
            cur["nt"] += nt
        if cur is not None:
            ops.append(cur)

    # per-core idx/dstl arrays
    idx_arrs, dstl_arrs = [], []
    for c in range(NCORES):
        key_s, lidx_s, dl_s = percore[c]
        gstart = np.concatenate([[0], np.cumsum(cnts[c])[:-1]])
        within = np.arange(len(key_s)) - gstart[key_s]
        pos = off_slot[key_s] + within
        idx_a = np.full(S_total, -1, np.int16)
        dstl_a = np.full(S_total, -1.0, np.float32)
        idx_a[pos] = lidx_s.astype(np.int16)
        dstl_a[pos] = dl_s
        # interior buckets of each op: pads -> 0 (safe row); last bucket
        # keeps -1 so the ucode trims trailing descriptors.
        for op in ops:
            for (kk, _, _) in op["buckets"][:-1]:
                s0, s1 = int(off_slot[kk]), int(off_slot[kk] + S_g[kk])
                seg = idx_a[s0:s1]
                seg[seg < 0] = 0
        if not TRIM:
            idx_a[idx_a < 0] = 0
        idx_arrs.append(_wrap16(idx_a))
        dstl_arrs.append(dstl_a.reshape(-1, 128).T.astype(_BF16))

    # MLP inputs (transposed x per core)
    xT = np.zeros((N_IN, NCORES * RPAD), _BF16)
    xv = x.astype(_BF16)
    for c in range(NCORES):
        xT[:, c * RPAD:c * RPAD + RPC] = xv[c * RPC:(c + 1) * RPC].T
    w1t = np.ascontiguousarray(W1.T).astype(_BF16)
    w2t = np.ascontiguousarray(W2.T).astype(_BF16)
    KH = N_HID // 128
    b1c = np.ascontiguousarray(b1.reshape(KH, 128).T).astype(np.float32)
    b2b01 = np.tile(b2.astype(np.float32) * ALPHA, (128, 1)).astype(np.float32)

    dinv_pad = np.ones(NCORES * RPAD, np.float32)
    for c in range(NCORES):
        dinv_pad[c * RPAD:c * RPAD + RPC] = dinv[c * RPC:(c + 1) * RPC]
    dnv = dinv_pad.reshape(NCORES, NB, 128)
    scu = ((1.0 - ALPHA) * dnv * dnv).astype(np.float32)
    scz = ((1.0 - ALPHA) * dnv).astype(np.float32)

    iota8 = np.broadcast_to(np.arange(128, dtype=np.float32), (128, 8, 128))

    meta = dict(N=N, N_IN=N_IN, N_HID=N_HID, F=F, RPC=RPC, NB=NB, RPAD=RPAD,
                QROWS=QROWS, S_total=S_total, T_total=T_total, ops=ops)
    in_maps = []
    for c in range(NCORES):
        in_maps.append({
            "xT": np.ascontiguousarray(xT[:, c * RPAD:(c + 1) * RPAD]),
            "w1t": w1t, "w2t": w2t, "b1c": b1c, "b2b01": b2b01,
            "idx": np.ascontiguousarray(idx_arrs[c]),
            "dstl": np.ascontiguousarray(dstl_arrs[c]),
            "scu": np.ascontiguousarray(scu[c].transpose(1, 0)),
            "scz": np.ascontiguousarray(scz[c].transpose(1, 0)),
            "dnv": np.ascontiguousarray(dnv[c].transpose(1, 0)),
            "iota8": np.ascontiguousarray(iota8.astype(_BF16)),
        })
    return meta, in_maps
